# revision 28
# baseline (speedup 1.0000x reference)
"""Multi-head attention (RMSNorm q/k, dense softmax) on 8 TRN2 NeuronCores.

Sharding: core c -> batch b = c//2, head-group g = c%2 (8 of 16 heads).
Each core computes a partial y.T = (Wo_g @ O_g.T) for its batch; the host
sums the two head-group partials per batch and transposes back (partials
ship as bf16; the host sum is fp32).

v9 (419us, from 456us): the ScalarE exp stream (256 x [128,1024] EXPs,
~1.11us each) is the serial floor; everything else hides under it:
  - sel matrices are host-built and weights ship in pre-tiled single-DMA
    layouts: SP DMA-trigger issue is ~640ns each and serializes, so the
    input set is 14 triggers ordered wk-h0, xT row-tiles, sel, wq-h0, wv
    (wo deferred into the stream at step 30);
  - prologue: the four K-side 512-token chunk chains run 4-way
    interleaved across PSUM accumulators (aux + st half-tiles) so all
    finish right as the last xT row-tile lands; q-chunk-0 chains behind;
    PSUM->SBUF copies on DVE with squares on the then-idle ScalarE;
  - rms rsqrt = DVE-only bit-trick seed (0x5f3759df with the /Dh folded
    into the constant) + one Newton step, bf16 out. No Ln/Exp ACT-table
    loads: v2 lost ~2.2us of exp-stream stall at each of 6 stage
    boundaries to table thrash;
  - remaining projection work (V tiles 4-15, hp0 q-chunks 1-3, the next
    head-pair QK per stage) drains from an ordered work queue at 3/2/1
    pieces per step; group-sum and rexp matmuls that consume DVE results
    are emitted via a deferred queue 2-4 steps later so they never block
    the in-order PE queue (keeping the PE dense also matters: idle gaps
    >3.4us re-throttle the clock gate to half rate);
  - hp3 interleaves the output projection for earlier token chunks at
    2 pieces per step plus a kt0-2 partial for the last token chunk, so
    the epilogue is 8 kt=3 matmuls + bf16 adds + DMA.
"""

import numpy as np
import ml_dtypes

B, N, D, H, Dh = 4, 2048, 1024, 16, 64
HPC = 8  # heads per core
GD = HPC * Dh  # 512 out-dims per core per projection
EPS = float(np.finfo(np.float32).eps)
NT = N // 128  # 16 token tiles
DT = D // 128  # 8 contraction tiles over D
KT = GD // 128  # 4 contraction tiles over the head-dim group
PTR = 16  # pt ring depth
PVL = 12  # PV software-pipeline lag (steps)

_NC_CACHE = {}


def _build_nc():
    import concourse.tile as tile
    from concourse import bacc, mybir

    f32 = mybir.dt.float32
    bf16 = mybir.dt.bfloat16
    u32 = mybir.dt.uint32
    i32 = mybir.dt.int32
    AF = mybir.ActivationFunctionType
    Alu = mybir.AluOpType

    nc = bacc.Bacc(None, target_bir_lowering=False)

    xT_e = nc.declare_dram_parameter("xT", [D, N], bf16, isOutput=False)
    wqT_e = nc.declare_dram_parameter("wqT", [128, DT, GD], bf16, isOutput=False)
    wkT_e = nc.declare_dram_parameter("wkT", [128, DT, GD], bf16, isOutput=False)
    wvT_e = nc.declare_dram_parameter("wvT", [128, DT, GD], bf16, isOutput=False)
    woT_e = nc.declare_dram_parameter("woT", [128, KT, D], bf16, isOutput=False)
    selq_e = nc.declare_dram_parameter("selq", [128, 128], bf16, isOutput=False)
    selk_e = nc.declare_dram_parameter("selk", [128, 128], bf16, isOutput=False)
    out_e = nc.declare_dram_parameter("out", [D, N], bf16, isOutput=True)

    RSK = 0x5F3759DF + 3 * (1 << 23)  # rsqrt seed const with /Dh folded in

    with nc.allow_low_precision(reason="f32r rounding / bf16 PV+out path"), \
            tile.TileContext(nc) as tc:
        from contextlib import ExitStack

        with ExitStack() as ctx:
            ep = ctx.enter_context
            consts = ep(tc.tile_pool(name="consts", bufs=1))
            xpool = ep(tc.tile_pool(name="x", bufs=1))
            wqp = ep(tc.tile_pool(name="wq", bufs=1))
            wkp = ep(tc.tile_pool(name="wk", bufs=1))
            wvp = ep(tc.tile_pool(name="wv", bufs=1))
            wop = ep(tc.tile_pool(name="wo", bufs=1))
            vpool = ep(tc.tile_pool(name="v", bufs=1))
            qknp = ep(tc.tile_pool(name="qkn", bufs=1))
            ptp = ep(tc.tile_pool(name="pt", bufs=1))
            otp = ep(tc.tile_pool(name="ot", bufs=1))
            scratch = ep(tc.tile_pool(name="scr", bufs=2))
            smallp = ep(tc.tile_pool(name="small", bufs=2))
            stp = ep(tc.tile_pool(name="st", bufs=1, space="PSUM"))
            pvp = ep(tc.tile_pool(name="pv", bufs=2, space="PSUM"))
            msp = ep(tc.tile_pool(name="ms", bufs=1, space="PSUM"))
            auxp = ep(tc.tile_pool(name="aux", bufs=1, space="PSUM"))

            # ---- DMAs: order = SP trigger issue order; keep the critical
            # path (wk, xt, wq) in front. wo is deferred into the stream.
            wkh0 = wkp.tile([128, DT, 128], bf16, name="wkh0")
            nc.sync.dma_start(wkh0[:], wkT_e[:, :, 0:128])
            xt = []
            for i in range(DT):
                t = xpool.tile([128, N], bf16, name=f"xt{i}")
                nc.sync.dma_start(t[:], xT_e[128 * i : 128 * (i + 1), :])
                xt.append(t)
            selk = consts.tile([128, 128], bf16, name="selk")
            nc.sync.dma_start(selk[:], selk_e[:, :])
            selq = consts.tile([128, 128], bf16, name="selq")
            nc.sync.dma_start(selq[:], selq_e[:, :])
            wqh0 = wqp.tile([128, DT, 128], bf16, name="wqh0")
            nc.sync.dma_start(wqh0[:], wqT_e[:, :, 0:128])
            wva = wvp.tile([128, DT, GD], bf16, name="wva")
            nc.sync.dma_start(wva[:], wvT_e[:, :, :])
            wka = wkp.tile([128, DT, GD - 128], bf16, name="wka")
            nc.sync.dma_start(wka[:], wkT_e[:, :, 128:GD])
            wqa = wqp.tile([128, DT, GD - 128], bf16, name="wqa")
            nc.sync.dma_start(wqa[:], wqT_e[:, :, 128:GD])
            woa = wop.tile([128, KT, D], bf16, name="woa")  # dma deferred

            # gones [128,32]: col h = head-h indicator; cols 2-31 zero so the
            # ms matmul zero-fills its whole 32-row output block.
            gones = consts.tile([128, 32], bf16)
            nc.vector.memset(gones[:], 0.0)
            nc.vector.memset(gones[0:64, 0:1], 1.0)
            nc.vector.memset(gones[64:128, 1:2], 1.0)

            ot = [otp.tile([128, N], bf16, name=f"ot{i}") for i in range(KT)]
            qkn_all = [
                [
                    qknp.tile([128, N], bf16, name=f"qkn{hp}_{side}")
                    for side in range(2)
                ]
                for hp in range(4)
            ]
            vsb = [
                vpool.tile([128, HPC, Dh + 1], bf16, name=f"v{tt}")
                for tt in range(NT)
            ]

            # ---- pslot: rotating PSUM chain accumulators ----
            slot_i = [0]
            pro_slots = [[(auxp, "aux", [128, 512])]]

            def pslot():
                rot = pro_slots[0]
                i = slot_i[0]
                slot_i[0] += 1
                pool, tag, shape = rot[i % len(rot)]
                return pool.tile(shape, f32, name=tag)

            # ---- DVE rsqrt: rinv = (ms/Dh)^-0.5, bf16 out ----
            def rms_rsqrt(ms_ap, rows, tag):
                del tag
                r0, r1 = rows
                h = scratch.tile([128, 512], u32, name="rsh", bufs=1)
                nc.vector.tensor_scalar(
                    h[r0:r1, :], ms_ap[r0:r1, :].bitcast(u32), 1, None,
                    Alu.logical_shift_right,
                )
                y0 = scratch.tile([128, 512], i32, name="rsy", bufs=1)
                nc.vector.tensor_scalar(
                    y0[r0:r1, :], h[r0:r1, :].bitcast(i32), -1, RSK,
                    Alu.mult, Alu.add,
                )
                y0f = y0.bitcast(f32)
                a = h.bitcast(f32)  # h is dead; reuse its buffer
                nc.vector.tensor_mul(a[r0:r1, :], y0f[r0:r1, :], y0f[r0:r1, :])
                b = scratch.tile([128, 512], f32, name="rsb", bufs=1)
                nc.vector.scalar_tensor_tensor(
                    b[r0:r1, :], a[r0:r1, :], -0.5 / Dh, ms_ap[r0:r1, :],
                    Alu.mult, Alu.mult,
                )
                rinv = scratch.tile([128, 512], bf16, name="rsr", bufs=2)
                nc.vector.scalar_tensor_tensor(
                    rinv[r0:r1, :], b[r0:r1, :], 1.5, y0f[r0:r1, :],
                    Alu.add, Alu.mult,
                )
                return rinv

            def rexp_mul(sel, rinv, dst, c4, slot=None):
                r0 = 32 * (c4 if slot is None else slot)
                rexp = pslot()
                nc.tensor.matmul(
                    rexp[:, 0:512],
                    sel[r0 : r0 + 2, :],
                    rinv[r0 : r0 + 2, :],
                    start=True,
                    stop=True,
                    tile_position=(r0, 0),
                )
                slc = dst[:, 512 * c4 : 512 * (c4 + 1)]
                nc.vector.tensor_mul(slc, slc, rexp[:, 0:512])

            # ---- projection chain pieces ----
            def vproj_gen(tt, scalar_copy=False):
                vps = pslot()
                for dt_ in range(0, DT, 2):
                    for d2 in (dt_, dt_ + 1):
                        nc.tensor.matmul(
                            vps[:, 0:512],
                            xt[d2][:, 128 * tt : 128 * (tt + 1)],
                            wva[:, d2, :],
                            start=(d2 == 0),
                            stop=(d2 == DT - 1),
                        )
                    if dt_ < DT - 2:
                        yield
                eng = nc.scalar if scalar_copy else nc.vector
                if scalar_copy:
                    eng.copy(
                        vsb[tt][:, :, 0:Dh],
                        vps[:, 0:512].rearrange("p (h d) -> p h d", h=HPC),
                    )
                else:
                    eng.tensor_copy(
                        vsb[tt][:, :, 0:Dh],
                        vps[:, 0:512].rearrange("p (h d) -> p h d", h=HPC),
                    )
                nc.vector.memset(vsb[tt][:, :, Dh : Dh + 1], 1.0)
                yield

            mside = {}

            def qk_chunk_gen(hp, side, c4, ms_row0=None):
                """Project one 512-token chunk, square, group-sum into the
                side's held ms PSUM tile at 32-aligned rows."""
                dst = qkn_all[hp][side]
                row0 = 32 * c4 if ms_row0 is None else ms_row0
                if (hp, side) not in mside:
                    mside[(hp, side)] = msp.tile([128, 512], f32, name="ms")
                qps = pslot()
                for dt_ in range(0, DT, 2):
                    for d2 in (dt_, dt_ + 1):
                        w_ap = (
                            (wqh0, wkh0)[side][:, d2, :]
                            if hp == 0
                            else (wqa, wka)[side][:, d2, 128 * (hp - 1) : 128 * hp]
                        )
                        nc.tensor.matmul(
                            qps[:, 0:512],
                            w_ap,
                            xt[d2][:, 512 * c4 : 512 * (c4 + 1)],
                            start=(d2 == 0),
                            stop=(d2 == DT - 1),
                        )
                    if dt_ < DT - 2:
                        yield
                sl = dst[:, 512 * c4 : 512 * (c4 + 1)]
                nc.vector.tensor_copy(sl, qps[:, 0:512])
                q2 = scratch.tile([128, 512], bf16, name="q2")
                nc.vector.tensor_mul(q2[:], sl, sl)
                mtile = mside[(hp, side)]

                def ms_mm(mtile=mtile, q2=q2, row0=row0):
                    nc.tensor.matmul(
                        mtile[row0 : row0 + 32, :],
                        gones[:],
                        q2[:],
                        start=True,
                        stop=True,
                        tile_position=(0, row0),
                    )

                defer(2, ms_mm)
                yield

            def qk_rms_gen(hp, side, chunks, tag):
                """DVE rsqrt over packed ms rows for `chunks`, then rexps."""
                sel = (selq, selk)[side]
                dst = qkn_all[hp][side]

                def do_rms():
                    mb = mside.pop((hp, side))
                    r0, r1 = 32 * chunks[0], 32 * (chunks[-1] + 1)
                    rinv = rms_rsqrt(mb[:], (r0, r1), tag)
                    for c4 in chunks:
                        defer(
                            2 + c4 // 2,
                            lambda c4=c4, rinv=rinv: rexp_mul(
                                sel, rinv, dst, c4
                            ),
                        )

                defer(4, do_rms)
                yield

            op3_parts = {}

            def outproj3_partial_gen(do):
                yps = pslot()
                for kt_ in range(3):
                    nc.tensor.matmul(
                        yps[:, 0:512],
                        woa[:, kt_, 128 * do : 128 * (do + 1)],
                        ot[kt_][:, 512 * 3 : 512 * 4],
                        start=(kt_ == 0),
                        stop=(kt_ == 2),
                    )
                    if kt_ == 1:
                        yield
                p = scratch.tile([128, 512], bf16, name="op3p", bufs=8)
                nc.vector.tensor_copy(p[:], yps[:, 0:512])
                op3_parts[do] = p
                yield

            def outproj3_final_gen():
                for do in range(DT):
                    yps = pslot()
                    nc.tensor.matmul(
                        yps[:, 0:512],
                        woa[:, 3, 128 * do : 128 * (do + 1)],
                        ot[3][:, 512 * 3 : 512 * 4],
                        start=True,
                        stop=True,
                    )
                    ysb = scratch.tile([128, 512], bf16, name="ysb", bufs=4)
                    nc.vector.tensor_add(ysb[:], yps[:, 0:512], op3_parts[do][:])
                    nc.sync.dma_start(
                        out_e[128 * do : 128 * (do + 1), 512 * 3 : 512 * 4],
                        ysb[:],
                    )
                    yield

            def outproj_gen(tch, do):
                yps = pslot()
                for kt_ in range(KT):
                    nc.tensor.matmul(
                        yps[:, 0:512],
                        woa[:, kt_, 128 * do : 128 * (do + 1)],
                        ot[kt_][:, 512 * tch : 512 * (tch + 1)],
                        start=(kt_ == 0),
                        stop=(kt_ == KT - 1),
                    )
                    if kt_ == 1:
                        yield
                ysb = scratch.tile([128, 512], bf16, name="ysb", bufs=4)
                nc.vector.tensor_copy(ysb[:], yps[:, 0:512])
                nc.sync.dma_start(
                    out_e[
                        128 * do : 128 * (do + 1),
                        512 * tch : 512 * (tch + 1),
                    ],
                    ysb[:],
                )
                yield

            pending = []  # deferred normalize tails (DVE muls)

            def drain_block(pvs, hp, qc):
                for side in range(2):
                    p0 = 64 * side
                    den0 = smallp.tile([1, 512], f32, name="den0", bufs=2)
                    nc.vector.tensor_copy(den0[:], pvs[side][Dh : Dh + 1, :])
                    rdenf = smallp.tile([1, 512], f32, name="rdenf", bufs=2)
                    nc.vector.reciprocal_approx_fast(rdenf[:], den0[:])
                    rde = scratch.tile([Dh, 512], f32, name="rde", bufs=3)
                    nc.gpsimd.partition_broadcast(rde[:], rdenf[:], channels=Dh)
                    oraw = scratch.tile([Dh + 1, 512], f32, name="oraw", bufs=3)
                    nc.vector.tensor_copy(oraw[:], pvs[side][:])
                    osl = ot[hp][p0 : p0 + 64, 512 * qc : 512 * (qc + 1)]

                    def fin(oraw=oraw, rde=rde, osl=osl):
                        nc.vector.tensor_mul(osl, oraw[0:Dh, :], rde[:])

                    pending.append(fin)

            # ---- attention step machinery ----
            steps = [
                (hp, qc, j) for hp in range(4) for qc in range(4)
                for j in range(NT)
            ]
            n = len(steps)
            blk_pvs = {}
            sts = {}
            pts = {}

            def emit_st(k):
                hp, qc, j = steps[k]
                qn, kn = qkn_all[hp]
                st = stp.tile([128, 1024], f32, name=f"st{k % 2}", bufs=1)
                for side in range(2):
                    p0 = 64 * side
                    nc.tensor.matmul(
                        st[:, 512 * side : 512 * (side + 1)],
                        kn[p0 : p0 + 64, 128 * j : 128 * (j + 1)],
                        qn[p0 : p0 + 64, 512 * qc : 512 * (qc + 1)],
                        start=True,
                        stop=True,
                    )
                sts[k] = st

            def emit_exp(k):
                pt = ptp.tile([128, 1024], bf16, name=f"pt{k % PTR}", bufs=1)
                nc.scalar.activation(pt[:], sts.pop(k)[:], AF.Exp, scale=Dh**-0.5)
                pts[k] = pt

            def emit_pv(k):
                hp, qc, j = steps[k]
                if j == 0:
                    blk_pvs[(hp, qc)] = [
                        pvp.tile([Dh + 1, 512], f32, name="pv") for _ in range(2)
                    ]
                pvs = blk_pvs[(hp, qc)]
                pt = pts.pop(k)
                for side in range(2):
                    nc.tensor.matmul(
                        pvs[side][:],
                        vsb[j][:, 2 * hp + side, :],
                        pt[:, 512 * side : 512 * (side + 1)],
                        start=(j == 0),
                        stop=(j == NT - 1),
                    )
                if j == NT - 1:
                    drain_block(blk_pvs.pop((hp, qc)), hp, qc)
                if j == 2 and pending:
                    for fn in pending:
                        fn()
                    pending.clear()

            # ---- prologue ----
            def drain(g):
                for _ in g:
                    pass

            # K chunk chains 4-way interleaved so each completes as soon as
            # the last xt row-tile lands; accumulators: aux + st0/st1 halves.
            aux0 = auxp.tile([128, 512], f32, name="aux")
            st0p = stp.tile([128, 1024], f32, name="st0")
            st1p = stp.tile([128, 1024], f32, name="st1")
            kaccs = [
                aux0[:, 0:512],
                st0p[:, 0:512],
                st0p[:, 512:1024],
                st1p[:, 0:512],
            ]
            for dt_ in range(DT):
                for c in range(4):
                    nc.tensor.matmul(
                        kaccs[c],
                        wkh0[:, dt_, :],
                        xt[dt_][:, 512 * c : 512 * (c + 1)],
                        start=(dt_ == 0),
                        stop=(dt_ == DT - 1),
                    )
            # q chunk 0 chained into st1's upper half right behind
            qacc = st1p[:, 512:1024]
            for dt_ in range(DT):
                nc.tensor.matmul(
                    qacc,
                    wqh0[:, dt_, :],
                    xt[dt_][:, 0:512],
                    start=(dt_ == 0),
                    stop=(dt_ == DT - 1),
                )
            # copies on DVE, squares on ScalarE (pipelined); ms group-sums
            kms = msp.tile([128, 512], f32, name="ms")
            kn0 = qkn_all[0][1]
            qn0 = qkn_all[0][0]
            q2s = []
            for c in range(4):
                sl = kn0[:, 512 * c : 512 * (c + 1)]
                nc.vector.tensor_copy(sl, kaccs[c])
                q2 = scratch.tile([128, 512], bf16, name="q2")
                nc.scalar.square(q2[:], sl)
                q2s.append(q2)
            nc.vector.tensor_copy(qn0[:, 0:512], qacc)
            q2q = scratch.tile([128, 512], bf16, name="q2")
            nc.scalar.square(q2q[:], qn0[:, 0:512])
            for c in range(4):
                nc.tensor.matmul(
                    kms[32 * c : 32 * c + 32, :],
                    gones[:],
                    q2s[c][:],
                    start=True,
                    stop=True,
                    tile_position=(0, 32 * c),
                )
            qms = msp.tile([128, 512], f32, name="ms")
            nc.tensor.matmul(
                qms[0:32, :], gones[:], q2q[:], start=True, stop=True,
                tile_position=(0, 0),
            )
            # V tiles 0-3 on the PE while the rms chains run on DVE
            pro_slots[0] = [(auxp, "aux", [128, 512]), (stp, "st1", [128, 1024])]
            vgens = [vproj_gen(tt, scalar_copy=True) for tt in range(4)]
            rinvk = rms_rsqrt(kms[:], (0, 128), "k0")
            rinvq0 = rms_rsqrt(qms[:], (0, 32), "q0")
            drain(vgens[0])
            drain(vgens[1])
            rexp_mul(selq, rinvq0, qn0, 0)
            for c in range(4):
                rexp_mul(selk, rinvk, kn0, c)
            drain(vgens[2])
            drain(vgens[3])
            pro_slots[0] = [(auxp, "aux", [128, 512])]

            # ---- stream work queue ----
            # Piece cost is roughly one 2-matmul chain segment (~0.43us PE);
            # quotas keep each step's PE under the ~1.09us exp pace.
            work_q = []

            def q_chunk_hp0_gen(c):
                # chunks 1-3 share one ms tile at 32-row slots 0-2; rms per
                # chunk reads its 32-row slot (offset reads <=32 partitions
                # are legal) and is deferred past the deferred ms matmul.
                if (0, 0) not in mside:
                    mside[(0, 0)] = msp.tile([128, 512], f32, name="ms")
                mb = mside[(0, 0)]
                yield from qk_chunk_gen(0, 0, c, ms_row0=32 * (c - 1))

                def do_rms(c=c, mb=mb):
                    r0 = 32 * (c - 1)
                    rinv = rms_rsqrt(mb[:], (r0, r0 + 32), "q")
                    defer(
                        2,
                        lambda: rexp_mul(
                            selq, rinv, qkn_all[0][0], c, slot=c - 1
                        ),
                    )

                defer(4, do_rms)
                yield

            def stage_gen(nhp):
                for side in range(2):
                    for c in range(4):
                        yield from qk_chunk_gen(nhp, side, c)
                    yield from qk_rms_gen(nhp, side, [0, 1, 2, 3], f"s{nhp}{side}")

            work_q.append(q_chunk_hp0_gen(1))
            for tt in range(4, 9):
                work_q.append(vproj_gen(tt))
            work_q.append(q_chunk_hp0_gen(2))
            for tt in range(9, 13):
                work_q.append(vproj_gen(tt))
            work_q.append(q_chunk_hp0_gen(3))
            for tt in range(13, NT):
                work_q.append(vproj_gen(tt))
            for nhp in (1, 2, 3):
                work_q.append(stage_gen(nhp))

            def drain_quota(quota):
                while quota > 0 and work_q:
                    if next(work_q[0], StopIteration) is StopIteration:
                        work_q.pop(0)
                        continue
                    quota -= 1

            cur_k = [0]
            defer_q = []  # (ready_step, closure), kept sorted by append order

            def defer(lag, fn):
                defer_q.append((cur_k[0] + lag, fn))

            def run_deferred():
                while defer_q and defer_q[0][0] <= cur_k[0]:
                    defer_q.pop(0)[1]()

            # ---- hp3 hooks: output projection interleave (kp-keyed) ----
            hooks2 = {}
            for qc in range(1, 4):
                for i8, jj in enumerate(range(3, 11)):
                    hooks2.setdefault((3, qc, jj), []).append(
                        outproj_gen(qc - 1, i8)
                    )
            part_slots = [
                (3, 1, 11), (3, 1, 12), (3, 1, 13), (3, 1, 14),
                (3, 2, 11), (3, 2, 12), (3, 2, 13), (3, 2, 14),
            ]
            for do, slot in enumerate(part_slots):
                hooks2.setdefault(slot, []).append(outproj3_partial_gen(do))

            # ---- main loop ----
            hook_q = []
            for k in range(n + PVL + 1):
                cur_k[0] = k
                run_deferred()
                if k == 30:
                    nc.sync.dma_start(woa[:], woT_e[:, :, :])
                # PV first: it executes while exp(k-2) finishes, so ST(k)'s
                # st-ring WAR wait is satisfied at decode (no pipeline break)
                kp = k - PVL - 1
                if 0 <= kp < n:
                    if kp == 192:
                        pro_slots[0] = [
                            (auxp, "aux", [128, 512]),
                            (msp, "ms", [128, 512]),
                        ]
                    emit_pv(kp)
                if k < n:
                    emit_st(k)
                    if k < 27:
                        drain_quota(3)
                    elif k < 40:
                        drain_quota(2)
                    else:
                        drain_quota(1)
                if 0 < k <= n:
                    emit_exp(k - 1)
                if 0 <= kp < n:
                    hook_q.extend(hooks2.get(steps[kp], []))
                    quota = 2
                    while quota > 0 and hook_q:
                        if next(hook_q[0], StopIteration) is StopIteration:
                            hook_q.pop(0)
                            continue
                        quota -= 1
            while hook_q:
                drain(hook_q.pop(0))
            while work_q:
                drain_quota(1000)
            for _, fn in defer_q:
                fn()
            defer_q.clear()
            for fn in pending:
                fn()
            pro_slots[0] = [
                (auxp, "aux", [128, 512]),
                (msp, "ms", [128, 512]),
                (stp, "st0", [128, 1024]),
                (stp, "st1", [128, 1024]),
            ]
            drain(outproj3_final_gen())

    nc.compile()
    return nc


def _get_nc():
    if "nc" not in _NC_CACHE:
        _NC_CACHE["nc"] = _build_nc()
    return _NC_CACHE["nc"]


def _tile_rows(w, nt):
    # [nt*128, F] -> [128, nt, F] contiguous
    f = w.shape[1]
    return np.ascontiguousarray(
        w.reshape(nt, 128, f).transpose(1, 0, 2)
    )


def make_in_maps(x, Wq, Wk, Wv, Wo, qn_w, kn_w):
    x = np.asarray(x, np.float32)
    Wq, Wk, Wv, Wo = (np.asarray(w, np.float32) for w in (Wq, Wk, Wv, Wo))
    qn_w = np.asarray(qn_w, np.float32).reshape(Dh)
    kn_w = np.asarray(kn_w, np.float32).reshape(Dh)
    selq = np.zeros((128, 128), np.float32)
    selk = np.zeros((128, 128), np.float32)
    for c in range(4):
        selq[32 * c, 0:64] = qn_w
        selq[32 * c + 1, 64:128] = qn_w
        selk[32 * c, 0:64] = kn_w
        selk[32 * c + 1, 64:128] = kn_w
    selq = selq.astype(ml_dtypes.bfloat16)
    selk = selk.astype(ml_dtypes.bfloat16)
    in_maps = []
    for c in range(8):
        b, g = c // 2, c % 2
        sl = slice(GD * g, GD * (g + 1))
        in_maps.append(
            {
                "xT": np.ascontiguousarray(x[b].T).astype(ml_dtypes.bfloat16),
                "wqT": _tile_rows(
                    np.ascontiguousarray(Wq[sl, :].T), DT
                ).astype(ml_dtypes.bfloat16),
                "wkT": _tile_rows(
                    np.ascontiguousarray(Wk[sl, :].T), DT
                ).astype(ml_dtypes.bfloat16),
                "wvT": _tile_rows(
                    np.ascontiguousarray(Wv[sl, :].T), DT
                ).astype(ml_dtypes.bfloat16),
                "woT": _tile_rows(
                    np.ascontiguousarray(Wo[:, sl].T), KT
                ).astype(ml_dtypes.bfloat16),
                "selq": selq,
                "selk": selk,
            }
        )
    return in_maps


def assemble(results):
    out = np.empty((B, N, D), np.float32)
    for b in range(B):
        out[b] = (
            results[2 * b]["out"].astype(np.float32)
            + results[2 * b + 1]["out"].astype(np.float32)
        ).T
    return out


def kernel(x, Wq, Wk, Wv, Wo, qn_w, kn_w):
    from concourse.bass_utils import run_bass_kernel_spmd

    nc = _get_nc()
    in_maps = make_in_maps(x, Wq, Wk, Wv, Wo, qn_w, kn_w)
    res = run_bass_kernel_spmd(nc, in_maps, core_ids=list(range(8)))
    return assemble(res.results)
